# revision 2
# baseline (speedup 1.0000x reference)
"""Batched cosine-similarity matrix (retrieval_knn) on 8 TRN2 NeuronCores.

reference:  out[b, n, m] = <x[b,n,:], y[b,m,:]> / max(||x[b,n]|| * ||y[b,m]||, 1e-8)
shapes:     x, y: [8, 2048, 512] f32  ->  out: [8, 2048, 2048] f32

Sharding: data-parallel over the batch dim - batch b runs on core b.

v2 design (vs the f32r v1 baseline at ~133us):
  - All-bf16 data path. Host casts inputs to bf16 (rel err ~2.5e-3 vs the
    2e-2 gate, verified numerically) which halves input DMA, guarantees
    1 cyc/row matmuls, and enables FWL weight loads. Output is written
    bf16 (8MB instead of 16MB) and upcast on host.
  - Inputs land via plain sync-engine HWDGE DMAs (contiguous 512KB
    blocks) instead of v1's slow gpsimd SWDGE cast path that kept the
    PE waiting 13us and HAM-throttled for the first 75us.
  - x row-norms: ACT square+accumulate over a host-packed natural-layout
    copy of x ([128, 16*512], tile t in cols 512t:512t+512) - zero PE
    and zero DVE cost (v1 spent ~5us of PE on 64 N=1 matmuls).
  - y col-norms: ones.T @ ysq in bf16, k-outer accumulation into 4 PSUM
    banks so the 16 matmuls run as y chunks land during the load phase.
  - 1/sqrt via exp(-0.5*ln(ssq)) on ACT (table accuracy ~1e-6, verified
    on HW) - v1's DVE reciprocal cost 3.4us per [128,512] chunk.
  - Warm-up dummy matmuls at t=0 get the PE HAM un-throttled (4/8 ->
    8/8) during the input load instead of oscillating mid-kernel.
  - Epilogue: single fused DVE scalar_tensor_tensor per [128,512] tile,
    staged into [128, 2048] rows, 512KB contiguous output DMAs on the
    ACT queue (sync queue is busy issuing input loads).
"""

import numpy as np
import ml_dtypes

import concourse.bass as bass
import concourse.bacc as bacc
import concourse.mybir as mybir
import concourse.tile as tile
from concourse.bass_utils import run_bass_kernel_spmd

P = 128          # partitions
D = 512          # feature dim (contraction)
N = 2048         # rows of x / y
B = 8            # batch == n_cores
KC = D // P      # 4 k-chunks
NT = N // P      # 16 n-tiles (output partition tiles)
MC = N // 512    # 4 m-chunks (PSUM-bank width)
WARMUP = 36      # dummy 128x128 matmuls to flip HAM to 8/8 (~3.8us @1.2GHz)

F32 = mybir.dt.float32
BF16 = mybir.dt.bfloat16

_CACHED = {}


def _build_nc(variant: str = "v2") -> bass.Bass:
    """Build the single-core Bass program (same program runs SPMD on 8 cores)."""
    nc = bacc.Bacc(trn_type="TRN2", target_bir_lowering=False, debug=False)

    xT = nc.dram_tensor("xT", [D, N], BF16, kind="ExternalInput").ap()
    yT = nc.dram_tensor("yT", [D, N], BF16, kind="ExternalInput").ap()
    # x in natural layout, tile-packed: xn[p, 512*t + d] = x[128*t + p, d]
    xn = nc.dram_tensor("xn", [P, NT * D], BF16, kind="ExternalInput").ap()
    out = nc.dram_tensor("out", [N, N], BF16, kind="ExternalOutput").ap()

    with tile.TileContext(nc) as tc:
        with (
            tc.tile_pool(name="consts", bufs=1) as const_pool,
            tc.tile_pool(name="xin", bufs=1) as xin_pool,
            tc.tile_pool(name="yin", bufs=1) as yin_pool,
            tc.tile_pool(name="ysqp", bufs=1) as ysq_pool,
            tc.tile_pool(name="xnat", bufs=1) as xn_pool,
            tc.tile_pool(name="sq", bufs=2) as sq_pool,
            tc.tile_pool(name="norms", bufs=1) as norm_pool,
            tc.tile_pool(name="ostage", bufs=3) as out_pool,
            tc.tile_pool(name="mm_ps", bufs=4, space="PSUM") as mm_ps_pool,
            tc.tile_pool(name="ry_ps", bufs=1, space="PSUM") as ry_ps_pool,
        ):
            ones = const_pool.tile([P, P], BF16, name="ones")
            nc.vector.memset(ones, 1.0)

            # ---- PE warm-up: junk matmuls with no input deps keep the
            # tensor engine busy from t=0 so HAM un-throttles during the
            # input load phase instead of oscillating mid-kernel.
            for w in range(WARMUP):
                wps = mm_ps_pool.tile([P, 512], F32, name="wps", tag="ps")
                nc.tensor.matmul(wps[:, 0:P], lhsT=ones, rhs=ones,
                                 start=True, stop=True)

            # ---- input loads: one contiguous 512KB DMA per [128, 2048]
            # chunk on the sync HWDGE queue, x/y interleaved so both are
            # available chunk-by-chunk for main matmuls and squares.
            xt, yt = [], []
            for k in range(KC):
                ks = slice(k * P, (k + 1) * P)
                xk = xin_pool.tile([P, N], BF16, name=f"xt{k}", tag=f"xt{k}")
                yk = yin_pool.tile([P, N], BF16, name=f"yt{k}", tag=f"yt{k}")
                nc.sync.dma_start(out=xk, in_=xT[ks, :])
                nc.sync.dma_start(out=yk, in_=yT[ks, :])
                xt.append(xk)
                yt.append(yk)

            # natural-layout x, 4 chunks of [128, 2048] (4 tiles each)
            xn_sb = xn_pool.tile([P, NT * D], BF16, name="xn_sb")
            for c in range(MC):
                cs = slice(c * N, (c + 1) * N)
                nc.sync.dma_start(out=xn_sb[:, cs], in_=xn[:, cs])

            # ---- y squares (ACT) as chunks land ---------------------
            ysq = []
            for k in range(KC):
                ys = ysq_pool.tile([P, N], BF16, name=f"ysq{k}", tag=f"ysq{k}")
                nc.scalar.square(ys, yt[k])
                ysq.append(ys)

            # ---- ssq_y: ones.T @ ysq, k-outer so each chunk's matmuls
            # issue as soon as its square is done; 4 PSUM banks accumulate.
            ry_ps = [
                ry_ps_pool.tile([P, 512], F32, name=f"ry_ps{c}", tag=f"ry_ps{c}")
                for c in range(MC)
            ]
            for k in range(KC):
                for c in range(MC):
                    cs = slice(c * 512, (c + 1) * 512)
                    nc.tensor.matmul(
                        ry_ps[c], lhsT=ones, rhs=ysq[k][:, cs],
                        start=(k == 0), stop=(k == KC - 1),
                    )

            # ---- ssq_x: ACT square with free-dim accumulate, one pass
            # per natural tile -> ssqx[:, t] = sum_d x[128t+p, d]^2.
            ssqx = norm_pool.tile([P, NT], F32, name="ssqx")
            for t in range(NT):
                scr = sq_pool.tile([P, D], BF16, name="sq_scr", tag="sq_scr")
                nc.scalar.activation(
                    scr, xn_sb[:, t * D:(t + 1) * D],
                    mybir.ActivationFunctionType.Square,
                    accum_out=ssqx[:, t:t + 1],
                )

            # ---- 1/sqrt on ACT via exp(-0.5*ln(s)); Rsqrt/Reciprocal
            # activations are banned and DVE reciprocal costs 6.4 cyc/elem.
            lnx = norm_pool.tile([P, NT], F32, name="lnx")
            rx = norm_pool.tile([P, NT], F32, name="rx")
            nc.scalar.activation(lnx, ssqx, mybir.ActivationFunctionType.Ln)
            nc.scalar.activation(rx, lnx, mybir.ActivationFunctionType.Exp,
                                 scale=-0.5)

            lny = norm_pool.tile([P, N], F32, name="lny")
            ry = norm_pool.tile([P, N], F32, name="ry")
            for c in range(MC):
                cs = slice(c * 512, (c + 1) * 512)
                nc.scalar.activation(lny[:, cs], ry_ps[c],
                                     mybir.ActivationFunctionType.Ln)
                nc.scalar.activation(ry[:, cs], lny[:, cs],
                                     mybir.ActivationFunctionType.Exp,
                                     scale=-0.5)

            # ---- main matmuls + fused epilogue ------------------------
            # k-inner accumulation into one PSUM bank per (t, c); DVE
            # drains bank i while PE fills i+1 (fill 853ns > drain 660ns).
            for t in range(NT):
                ts_ = slice(t * P, (t + 1) * P)
                ot = out_pool.tile([P, N], BF16, name="ot", tag="ot")
                for c in range(MC):
                    cs = slice(c * 512, (c + 1) * 512)
                    ps = mm_ps_pool.tile([P, 512], F32, name="ps", tag="ps")
                    for k in range(KC):
                        nc.tensor.matmul(
                            ps, lhsT=xt[k][:, ts_], rhs=yt[k][:, cs],
                            start=(k == 0), stop=(k == KC - 1),
                        )
                    # ot = (ps * rx[:, t]) * ry[:, chunk c]
                    nc.vector.scalar_tensor_tensor(
                        ot[:, cs], in0=ps, scalar=rx[:, t:t + 1],
                        in1=ry[:, cs],
                        op0=mybir.AluOpType.mult, op1=mybir.AluOpType.mult,
                    )
                # contiguous 512KB row-block store on the ACT HWDGE queue
                nc.scalar.dma_start(out=out[ts_, :], in_=ot)

    nc.compile()
    return nc


def _get_nc(variant: str = "v2") -> bass.Bass:
    if variant not in _CACHED:
        _CACHED[variant] = _build_nc(variant)
    return _CACHED[variant]


def _shard(x: np.ndarray, y: np.ndarray):
    """Host-side prep: cast to bf16, transpose to [512, 2048], and pack a
    natural-layout copy of x for the ACT norm path."""
    xq = np.asarray(x, dtype=np.float32).astype(ml_dtypes.bfloat16)
    yq = np.asarray(y, dtype=np.float32).astype(ml_dtypes.bfloat16)
    xTs = np.ascontiguousarray(np.transpose(xq, (0, 2, 1)))
    yTs = np.ascontiguousarray(np.transpose(yq, (0, 2, 1)))
    # xn[b, p, 512*t + d] = x[b, 128*t + p, d]
    xns = np.ascontiguousarray(
        xq.reshape(B, NT, P, D).transpose(0, 2, 1, 3).reshape(B, P, NT * D)
    )
    return [{"xT": xTs[b], "yT": yTs[b], "xn": xns[b]} for b in range(B)]


def _run(x: np.ndarray, y: np.ndarray, variant: str = "v2",
         trace: bool = False):
    """Returns (out [8, 2048, 2048] f32, BassKernelResults)."""
    nc = _get_nc(variant)
    in_maps = _shard(x, y)
    res = run_bass_kernel_spmd(nc, in_maps, core_ids=list(range(B)), trace=trace)
    out = np.stack([res.results[b]["out"].astype(np.float32) for b in range(B)])
    return out, res


def kernel(x: np.ndarray, y: np.ndarray) -> np.ndarray:
    out, _ = _run(x, y)
    return out


# revision 4
# speedup vs baseline: 1.0866x; 1.0866x over previous
"""Batched cosine-similarity matrix (retrieval_knn) on 8 TRN2 NeuronCores.

reference:  out[b, n, m] = <x[b,n,:], y[b,m,:]> / max(||x[b,n]|| * ||y[b,m]||, 1e-8)
shapes:     x, y: [8, 2048, 512] f32  ->  out: [8, 2048, 2048] f32

Sharding: data-parallel over the batch dim - batch b runs on core b.

v4 (v1 f32r 133us, v2 all-bf16 106us, v3 95us-but-broken):
  - All-bf16 data path; 4MB input (xT/yT only), bf16 output upcast on host.
  - v3's rx bug: interleaved per-column PSUM accumulation groups - a
    start=True matmul clears has_written for its whole BANK, so later
    columns' k=0 starts wiped earlier columns' accumulation state and
    every ssq_x lost its k=0 term (the measured 14% error = missing 1/4
    of the sum). v4 uses one rotating PSUM tile per k with
    start=stop=True on every N=1 matmul and accumulates on the DVE
    (idle during the load) into SBUF.
  - x loads fully before y: the rx chain (squares -> 64 N=1 MMs -> DVE
    adds -> Ln/Exp) completes by ~19us, before the first PSUM drain
    needs it; the ry chain only needs y + 0.9us of matmuls + per-chunk
    Ln/Exp pairs that pipeline just ahead of the drain order.
  - Squares on DVE (tensor_tensor mult, 2x bf16 mode) instead of ACT so
    the ACT table slots only ever hold Ln/Exp - v3 spent 9us reloading
    activation tables on the critical norm tail.
  - ry: ones.T @ ysq k-outer into 4 PSUM banks (single pass); main loop
    gets 3 rotating banks (fill 853ns > drain 750ns, 2.6us of buffer).
  - Dummy warm-up matmuls bridge PE waits during the load so HAM stays
    at K=8/8 for the 54.6us main MM stream.
"""

import numpy as np
import ml_dtypes

import concourse.bass as bass
import concourse.bacc as bacc
import concourse.mybir as mybir
import concourse.tile as tile
from concourse.bass_utils import run_bass_kernel_spmd

P = 128          # partitions
D = 512          # feature dim (contraction)
N = 2048         # rows of x / y
B = 8            # batch == n_cores
KC = D // P      # 4 k-chunks
NT = N // P      # 16 n-tiles (output partition tiles)
MC = N // 512    # 4 m-chunks (PSUM-bank width)
WARMUP = 36      # initial dummy matmuls (~3.8us @1.2GHz) to flip HAM to 8/8
FILL = 8         # dummy matmuls interleaved per k-group to bridge chunk waits

F32 = mybir.dt.float32
BF16 = mybir.dt.bfloat16

_CACHED = {}


def _build_nc(variant: str = "v4") -> bass.Bass:
    """Build the single-core Bass program (same program runs SPMD on 8 cores)."""
    nc = bacc.Bacc(trn_type="TRN2", target_bir_lowering=False, debug=False)

    xT = nc.dram_tensor("xT", [D, N], BF16, kind="ExternalInput").ap()
    yT = nc.dram_tensor("yT", [D, N], BF16, kind="ExternalInput").ap()
    out = nc.dram_tensor("out", [N, N], BF16, kind="ExternalOutput").ap()

    Ln = mybir.ActivationFunctionType.Ln
    Exp = mybir.ActivationFunctionType.Exp

    with tile.TileContext(nc) as tc:
        with (
            tc.tile_pool(name="consts", bufs=1) as const_pool,
            tc.tile_pool(name="xin", bufs=1) as xin_pool,
            tc.tile_pool(name="yin", bufs=1) as yin_pool,
            tc.tile_pool(name="sq", bufs=1) as sq_pool,
            tc.tile_pool(name="norms", bufs=1) as norm_pool,
            tc.tile_pool(name="ostage", bufs=3) as out_pool,
            tc.tile_pool(name="mm_ps", bufs=3, space="PSUM") as mm_ps_pool,
            tc.tile_pool(name="ry_ps", bufs=1, space="PSUM") as ry_ps_pool,
            tc.tile_pool(name="rx_ps", bufs=1, space="PSUM") as rx_ps_pool,
        ):
            ones = const_pool.tile([P, P], BF16, name="ones")
            nc.vector.memset(ones, 1.0)

            def dummy_mms(n):
                # junk matmuls with no input deps; they run whenever the
                # PE would otherwise idle waiting on a DMA chunk, keeping
                # the HAM activity window busy (no K=4/8 re-throttle).
                for _ in range(n):
                    wps = mm_ps_pool.tile([P, 512], F32, name="wps", tag="ps")
                    nc.tensor.matmul(wps[:, 0:P], lhsT=ones, rhs=ones,
                                     start=True, stop=True)

            dummy_mms(WARMUP)

            # ---- input loads: x fully first (rx chain is the long pole
            # and gates every epilogue), then y. 512KB contiguous DMAs.
            xt, yt = [], []
            for k in range(KC):
                xk = xin_pool.tile([P, N], BF16, name=f"xt{k}", tag=f"xt{k}")
                nc.sync.dma_start(out=xk, in_=xT[k * P:(k + 1) * P, :])
                xt.append(xk)
            for k in range(KC):
                yk = yin_pool.tile([P, N], BF16, name=f"yt{k}", tag=f"yt{k}")
                nc.sync.dma_start(out=yk, in_=yT[k * P:(k + 1) * P, :])
                yt.append(yk)

            # ---- squares on DVE (idle during load; keeps ACT tables
            # free for Ln/Exp only).
            xsq, ysq = [], []
            for k in range(KC):
                xs = sq_pool.tile([P, N], BF16, name=f"xsq{k}", tag=f"xsq{k}")
                nc.vector.tensor_tensor(xs, xt[k], xt[k], mybir.AluOpType.mult)
                xsq.append(xs)

            # ---- rx: ssq_x[n] via N=1 matmuls; per-k PSUM tile (every
            # matmul start=stop=True - bank-level has_written makes
            # cross-matmul PSUM accumulation unsafe here), DVE-accumulated
            # into SBUF. k-grouped so chunk k's matmuls run as xsq[k]
            # lands; dummies bridge the chunk waits.
            ssqx = norm_pool.tile([P, NT], F32, name="ssqx")
            for k in range(KC):
                rxk = rx_ps_pool.tile([P, NT], F32, name=f"rx_ps{k}", tag="rx")
                for t in range(NT):
                    nc.tensor.matmul(
                        rxk[:, t:t + 1],
                        lhsT=xsq[k][:, t * P:(t + 1) * P],
                        rhs=ones[:, 0:1],
                        start=True, stop=True,
                    )
                if k == 0:
                    nc.vector.tensor_copy(ssqx, rxk)
                else:
                    nc.vector.tensor_tensor(ssqx, ssqx, rxk,
                                            mybir.AluOpType.add)
                if k < KC - 1:
                    dummy_mms(FILL)

            # rx = exp(-0.5*ln(ssqx)): first ACT ops -> loads Ln then Exp
            # tables once; nothing else ever evicts them.
            lnx = norm_pool.tile([P, NT], F32, name="lnx")
            rx = norm_pool.tile([P, NT], F32, name="rx")
            nc.scalar.activation(lnx, ssqx, Ln)
            nc.scalar.activation(rx, lnx, Exp, scale=-0.5)

            # ---- y squares (DVE) + ry matmuls k-outer into 4 banks ----
            ry_ps = [
                ry_ps_pool.tile([P, 512], F32, name=f"ry_ps{c}", tag=f"ry{c}")
                for c in range(MC)
            ]
            for k in range(KC):
                ys = sq_pool.tile([P, N], BF16, name=f"ysq{k}", tag=f"ysq{k}")
                nc.vector.tensor_tensor(ys, yt[k], yt[k], mybir.AluOpType.mult)
                ysq.append(ys)
                for c in range(MC):
                    nc.tensor.matmul(
                        ry_ps[c], lhsT=ones, rhs=ys[:, c * 512:(c + 1) * 512],
                        start=(k == 0), stop=(k == KC - 1),
                    )
                if k < KC - 1:
                    dummy_mms(FILL)

            # ry = exp(-0.5*ln(ssqy)) per chunk, in drain order (c0 is
            # needed first); tables are already resident.
            lny = norm_pool.tile([P, N], F32, name="lny")
            ry = norm_pool.tile([P, N], F32, name="ry")
            for c in range(MC):
                cs = slice(c * 512, (c + 1) * 512)
                nc.scalar.activation(lny[:, cs], ry_ps[c], Ln)
                nc.scalar.activation(ry[:, cs], lny[:, cs], Exp, scale=-0.5)

            # ---- main matmuls + fused epilogue ------------------------
            # k-inner accumulation, 3 rotating PSUM banks; DVE drains bank
            # i (stt ~750ns) while the PE fills i+1 (853ns).
            for t in range(NT):
                ts_ = slice(t * P, (t + 1) * P)
                ot = out_pool.tile([P, N], BF16, name="ot", tag="ot")
                for c in range(MC):
                    cs = slice(c * 512, (c + 1) * 512)
                    ps = mm_ps_pool.tile([P, 512], F32, name="ps", tag="ps")
                    for k in range(KC):
                        nc.tensor.matmul(
                            ps, lhsT=xt[k][:, ts_], rhs=yt[k][:, cs],
                            start=(k == 0), stop=(k == KC - 1),
                        )
                    # ot = (ps * rx[:, t]) * ry[:, chunk c]
                    nc.vector.scalar_tensor_tensor(
                        ot[:, cs], in0=ps, scalar=rx[:, t:t + 1],
                        in1=ry[:, cs],
                        op0=mybir.AluOpType.mult, op1=mybir.AluOpType.mult,
                    )
                # contiguous 512KB row-block store on the ACT HWDGE queue
                nc.scalar.dma_start(out=out[ts_, :], in_=ot)

    nc.compile()
    return nc


def _get_nc(variant: str = "v4") -> bass.Bass:
    if variant not in _CACHED:
        _CACHED[variant] = _build_nc(variant)
    return _CACHED[variant]


def _shard(x: np.ndarray, y: np.ndarray):
    """Host-side prep: cast to bf16 and transpose to [512, 2048]."""
    xq = np.asarray(x, dtype=np.float32).astype(ml_dtypes.bfloat16)
    yq = np.asarray(y, dtype=np.float32).astype(ml_dtypes.bfloat16)
    xTs = np.ascontiguousarray(np.transpose(xq, (0, 2, 1)))
    yTs = np.ascontiguousarray(np.transpose(yq, (0, 2, 1)))
    return [{"xT": xTs[b], "yT": yTs[b]} for b in range(B)]


def _run(x: np.ndarray, y: np.ndarray, variant: str = "v4",
         trace: bool = False):
    """Returns (out [8, 2048, 2048] f32, BassKernelResults)."""
    nc = _get_nc(variant)
    in_maps = _shard(x, y)
    res = run_bass_kernel_spmd(nc, in_maps, core_ids=list(range(B)), trace=trace)
    out = np.stack([res.results[b]["out"].astype(np.float32) for b in range(B)])
    return out, res


def kernel(x: np.ndarray, y: np.ndarray) -> np.ndarray:
    out, _ = _run(x, y)
    return out


# revision 5
# speedup vs baseline: 1.1378x; 1.0472x over previous
"""Batched cosine-similarity matrix (retrieval_knn) on 8 TRN2 NeuronCores.

reference:  out[b, n, m] = <x[b,n,:], y[b,m,:]> / max(||x[b,n]|| * ||y[b,m]||, 1e-8)
shapes:     x, y: [8, 2048, 512] f32  ->  out: [8, 2048, 2048] f32

Sharding: data-parallel over the batch dim - batch b runs on core b.

v5 (v1 133us, v2 106us, v4 97.6us):
  - All-bf16 data path; 4MB input, bf16 output upcast on host.
  - Inputs split across BOTH HWDGE queues (x chunks on sync, y chunks
    on ACT), k-interleaved: v4 put all 8 DMAs on one queue at ~250GB/s
    effective, so y[3] landed at 24us and the y-norm chain pushed the
    first epilogue to 31us.
  - Deferred epilogue for the first 12 (t,c) groups: their PSUM banks
    drain with a plain f32 copy (392ns) into SBUF staging and the
    rx/ry scaling is applied later, once the norms exist. This takes
    the whole norm tail off the PE critical path - the PE streams
    matmuls back-to-back from the moment the load completes (12 groups
    x 853ns = 10us of slack vs a ~5us norm tail).
  - ACT has a single activation-table slot (v4 measured 6x 1.28us
    loads alternating Ln/Exp): batch all Lns then all Exps -> 2 loads.
  - rx: per-k N=1 matmuls with start=stop=True (PSUM has_written is
    bank-granular - cross-matmul accumulation groups interleaved on one
    bank lose terms), accumulated on the DVE into SBUF during the load.
  - Squares on DVE (idle during load) so ACT tables stay Ln/Exp only.
  - Dummy warm-up matmuls keep HAM at K=8/8 through the load phase.
"""

import numpy as np
import ml_dtypes

import concourse.bass as bass
import concourse.bacc as bacc
import concourse.mybir as mybir
import concourse.tile as tile
from concourse.bass_utils import run_bass_kernel_spmd

P = 128          # partitions
D = 512          # feature dim (contraction)
N = 2048         # rows of x / y
B = 8            # batch == n_cores
KC = D // P      # 4 k-chunks
NT = N // P      # 16 n-tiles (output partition tiles)
MC = N // 512    # 4 m-chunks (PSUM-bank width)
WARMUP = 36      # initial dummy matmuls (~3.8us @1.2GHz) to flip HAM to 8/8
FILL = 4         # dummy matmuls per k-group to bridge chunk waits
DEFER_T = 3      # tiles whose epilogue is deferred (12 groups of slack)

F32 = mybir.dt.float32
BF16 = mybir.dt.bfloat16

_CACHED = {}


def _build_nc(variant: str = "v5") -> bass.Bass:
    """Build the single-core Bass program (same program runs SPMD on 8 cores)."""
    nc = bacc.Bacc(trn_type="TRN2", target_bir_lowering=False, debug=False)

    xT = nc.dram_tensor("xT", [D, N], BF16, kind="ExternalInput").ap()
    yT = nc.dram_tensor("yT", [D, N], BF16, kind="ExternalInput").ap()
    out = nc.dram_tensor("out", [N, N], BF16, kind="ExternalOutput").ap()

    Ln = mybir.ActivationFunctionType.Ln
    Exp = mybir.ActivationFunctionType.Exp
    mult = mybir.AluOpType.mult

    with tile.TileContext(nc) as tc:
        with (
            tc.tile_pool(name="consts", bufs=1) as const_pool,
            tc.tile_pool(name="xin", bufs=1) as xin_pool,
            tc.tile_pool(name="yin", bufs=1) as yin_pool,
            tc.tile_pool(name="sq", bufs=1) as sq_pool,
            tc.tile_pool(name="norms", bufs=1) as norm_pool,
            tc.tile_pool(name="defer", bufs=1) as defer_pool,
            tc.tile_pool(name="ostage", bufs=3) as out_pool,
            tc.tile_pool(name="mm_ps", bufs=3, space="PSUM") as mm_ps_pool,
            tc.tile_pool(name="ry_ps", bufs=1, space="PSUM") as ry_ps_pool,
            tc.tile_pool(name="rx_ps", bufs=1, space="PSUM") as rx_ps_pool,
        ):
            ones = const_pool.tile([P, P], BF16, name="ones")
            nc.vector.memset(ones, 1.0)

            def dummy_mms(n):
                # junk matmuls with no input deps; they run whenever the
                # PE would otherwise idle waiting on a DMA chunk, keeping
                # the HAM activity window busy (no K=4/8 re-throttle).
                for _ in range(n):
                    wps = mm_ps_pool.tile([P, 512], F32, name="wps", tag="ps")
                    nc.tensor.matmul(wps[:, 0:P], lhsT=ones, rhs=ones,
                                     start=True, stop=True)

            dummy_mms(WARMUP)

            # ---- input loads: x on the sync queue, y on the ACT queue
            # (parallel DMA engines), k-interleaved 512KB contiguous DMAs.
            xt, yt = [], []
            for k in range(KC):
                xk = xin_pool.tile([P, N], BF16, name=f"xt{k}", tag=f"xt{k}")
                yk = yin_pool.tile([P, N], BF16, name=f"yt{k}", tag=f"yt{k}")
                nc.sync.dma_start(out=xk, in_=xT[k * P:(k + 1) * P, :])
                nc.scalar.dma_start(out=yk, in_=yT[k * P:(k + 1) * P, :])
                xt.append(xk)
                yt.append(yk)

            # ---- per-chunk load-phase work, k-grouped ----------------
            # squares on DVE; rx via N=1 matmuls (start=stop=True) into a
            # rotating per-k PSUM tile, DVE-accumulated into SBUF; ry via
            # ones.T @ ysq k-outer accumulating in 4 PSUM banks.
            ssqx = norm_pool.tile([P, NT], F32, name="ssqx")
            ry_ps = [
                ry_ps_pool.tile([P, 512], F32, name=f"ry_ps{c}", tag=f"ry{c}")
                for c in range(MC)
            ]
            for k in range(KC):
                xs = sq_pool.tile([P, N], BF16, name=f"xsq{k}", tag=f"xsq{k}")
                nc.vector.tensor_tensor(xs, xt[k], xt[k], mult)
                rxk = rx_ps_pool.tile([P, NT], F32, name=f"rx_ps{k}", tag="rx")
                for t in range(NT):
                    nc.tensor.matmul(
                        rxk[:, t:t + 1],
                        lhsT=xs[:, t * P:(t + 1) * P],
                        rhs=ones[:, 0:1],
                        start=True, stop=True,
                    )
                if k == 0:
                    nc.vector.tensor_copy(ssqx, rxk)
                else:
                    nc.vector.tensor_tensor(ssqx, ssqx, rxk,
                                            mybir.AluOpType.add)

                ys = sq_pool.tile([P, N], BF16, name=f"ysq{k}", tag=f"ysq{k}")
                nc.vector.tensor_tensor(ys, yt[k], yt[k], mult)
                for c in range(MC):
                    nc.tensor.matmul(
                        ry_ps[c], lhsT=ones, rhs=ys[:, c * 512:(c + 1) * 512],
                        start=(k == 0), stop=(k == KC - 1),
                    )
                if k < KC - 1:
                    dummy_mms(FILL)

            # ---- 1/sqrt via exp(-0.5*ln(s)); all Lns then all Exps so
            # the single ACT table slot loads each table once.
            lnx = norm_pool.tile([P, NT], F32, name="lnx")
            rx = norm_pool.tile([P, NT], F32, name="rx")
            lny = norm_pool.tile([P, N], F32, name="lny")
            ry = norm_pool.tile([P, N], F32, name="ry")
            nc.scalar.activation(lnx, ssqx, Ln)
            for c in range(MC):
                cs = slice(c * 512, (c + 1) * 512)
                nc.scalar.activation(lny[:, cs], ry_ps[c], Ln)
            nc.scalar.activation(rx, lnx, Exp, scale=-0.5)
            for c in range(MC):
                cs = slice(c * 512, (c + 1) * 512)
                nc.scalar.activation(ry[:, cs], lny[:, cs], Exp, scale=-0.5)

            # ---- main matmuls ----------------------------------------
            # k-inner accumulation, 3 rotating PSUM banks. First DEFER_T
            # tiles drain with a plain f32 copy into SBUF staging (392ns,
            # no rx/ry dependency) so the PE keeps streaming while the
            # norm tail completes; their scaling runs right after.
            ots = []
            stage = []
            for t in range(DEFER_T):
                ot = out_pool.tile([P, N], BF16, name="ot", tag="ot")
                ots.append(ot)
                for c in range(MC):
                    ps = mm_ps_pool.tile([P, 512], F32, name="ps", tag="ps")
                    for k in range(KC):
                        nc.tensor.matmul(
                            ps, lhsT=xt[k][:, t * P:(t + 1) * P],
                            rhs=yt[k][:, c * 512:(c + 1) * 512],
                            start=(k == 0), stop=(k == KC - 1),
                        )
                    st = defer_pool.tile([P, 512], F32, name=f"st{t}_{c}",
                                         tag=f"st{t}_{c}")
                    nc.vector.tensor_copy(st, ps)
                    stage.append((t, c, st))

            # deferred epilogues (wait on rx/ry, not on PSUM)
            for t, c, st in stage:
                cs = slice(c * 512, (c + 1) * 512)
                nc.vector.scalar_tensor_tensor(
                    ots[t][:, cs], in0=st, scalar=rx[:, t:t + 1],
                    in1=ry[:, cs], op0=mult, op1=mult,
                )
                if c == MC - 1:
                    nc.scalar.dma_start(out=out[t * P:(t + 1) * P, :],
                                        in_=ots[t])

            for t in range(DEFER_T, NT):
                ts_ = slice(t * P, (t + 1) * P)
                ot = out_pool.tile([P, N], BF16, name="ot", tag="ot")
                for c in range(MC):
                    cs = slice(c * 512, (c + 1) * 512)
                    ps = mm_ps_pool.tile([P, 512], F32, name="ps", tag="ps")
                    for k in range(KC):
                        nc.tensor.matmul(
                            ps, lhsT=xt[k][:, ts_], rhs=yt[k][:, cs],
                            start=(k == 0), stop=(k == KC - 1),
                        )
                    # ot = (ps * rx[:, t]) * ry[:, chunk c]
                    nc.vector.scalar_tensor_tensor(
                        ot[:, cs], in0=ps, scalar=rx[:, t:t + 1],
                        in1=ry[:, cs], op0=mult, op1=mult,
                    )
                # contiguous 512KB row-block store on the ACT HWDGE queue
                nc.scalar.dma_start(out=out[ts_, :], in_=ot)

    nc.compile()
    return nc


def _get_nc(variant: str = "v5") -> bass.Bass:
    if variant not in _CACHED:
        _CACHED[variant] = _build_nc(variant)
    return _CACHED[variant]


def _shard(x: np.ndarray, y: np.ndarray):
    """Host-side prep: cast to bf16 and transpose to [512, 2048]."""
    xq = np.asarray(x, dtype=np.float32).astype(ml_dtypes.bfloat16)
    yq = np.asarray(y, dtype=np.float32).astype(ml_dtypes.bfloat16)
    xTs = np.ascontiguousarray(np.transpose(xq, (0, 2, 1)))
    yTs = np.ascontiguousarray(np.transpose(yq, (0, 2, 1)))
    return [{"xT": xTs[b], "yT": yTs[b]} for b in range(B)]


def _run(x: np.ndarray, y: np.ndarray, variant: str = "v5",
         trace: bool = False):
    """Returns (out [8, 2048, 2048] f32, BassKernelResults)."""
    nc = _get_nc(variant)
    in_maps = _shard(x, y)
    res = run_bass_kernel_spmd(nc, in_maps, core_ids=list(range(B)), trace=trace)
    out = np.stack([res.results[b]["out"].astype(np.float32) for b in range(B)])
    return out, res


def kernel(x: np.ndarray, y: np.ndarray) -> np.ndarray:
    out, _ = _run(x, y)
    return out


# revision 7
# speedup vs baseline: 1.1612x; 1.0206x over previous
"""Batched cosine-similarity matrix (retrieval_knn) on 8 TRN2 NeuronCores.

reference:  out[b, n, m] = <x[b,n,:], y[b,m,:]> / max(||x[b,n]|| * ||y[b,m]||, 1e-8)
shapes:     x, y: [8, 2048, 512] f32  ->  out: [8, 2048, 2048] f32

Sharding: data-parallel over the batch dim - batch b runs on core b.

v6 (v1 133us, v2 106us, v4 97.6us, v5 93.2us):
  - All-bf16 data path; 4MB input split across both HWDGE queues
    (x on sync, y on ACT); bf16 output upcast on host.
  - v5's limiter was the DVE (busy 67us): every drain is a 750ns
    fp32-PSUM stt (PSUM source pins the DVE at 1x mode). v6 splits the
    epilogue: half the (t,c) groups drain on ACT - Copy(ps * rx) with a
    per-partition scale AP into bf16 - and finish with a 327ns bf16 2x
    DVE multiply by ry; the other half keep the single 750ns DVE stt.
    ~44us on each engine instead of 67 on one.
  - First 8 groups defer: plain DVE copy to SBUF staging (no rx/ry
    dependency), scaled later - the PE streams through the norm tail.
  - ry in bf16 (needed for the 2x DVE multiply; adds ~1e-3 rel err).
  - ACT's single table slot: all Lns then all Exps (2 loads + 1 for
    the hoisted first), after v4 measured 6 alternating reloads.
  - rx via per-k N=1 matmuls start=stop=True (PSUM has_written is
    bank-granular), DVE-accumulated; squares on DVE during the load.
  - Dummy warm-up matmuls bridge chunk waits (HAM stays at K=8/8).
"""

import numpy as np
import ml_dtypes

import concourse.bass as bass
import concourse.bacc as bacc
import concourse.mybir as mybir
import concourse.tile as tile
from concourse.bass_utils import run_bass_kernel_spmd

P = 128          # partitions
D = 512          # feature dim (contraction)
N = 2048         # rows of x / y
B = 8            # batch == n_cores
KC = D // P      # 4 k-chunks
NT = N // P      # 16 n-tiles (output partition tiles)
MC = N // 512    # 4 m-chunks (PSUM-bank width)
WARMUP = 36      # initial dummy matmuls (~3.8us @1.2GHz) to flip HAM to 8/8
FILL = 8         # dummy matmuls per k-group to bridge chunk waits
DEFER = 8        # groups drained unscaled to SBUF (PE slack over norm tail)

F32 = mybir.dt.float32
BF16 = mybir.dt.bfloat16

_CACHED = {}


def _build_nc(variant: str = "v6") -> bass.Bass:
    """Build the single-core Bass program (same program runs SPMD on 8 cores)."""
    nc = bacc.Bacc(trn_type="TRN2", target_bir_lowering=False, debug=False)

    xT = nc.dram_tensor("xT", [D, N], BF16, kind="ExternalInput").ap()
    yT = nc.dram_tensor("yT", [D, N], BF16, kind="ExternalInput").ap()
    out = nc.dram_tensor("out", [N, N], BF16, kind="ExternalOutput").ap()

    Ln = mybir.ActivationFunctionType.Ln
    Exp = mybir.ActivationFunctionType.Exp
    Copy = mybir.ActivationFunctionType.Copy
    mult = mybir.AluOpType.mult

    with tile.TileContext(nc) as tc:
        with (
            tc.tile_pool(name="consts", bufs=1) as const_pool,
            tc.tile_pool(name="xin", bufs=1) as xin_pool,
            tc.tile_pool(name="yin", bufs=1) as yin_pool,
            tc.tile_pool(name="sq", bufs=1) as sq_pool,
            tc.tile_pool(name="norms", bufs=1) as norm_pool,
            tc.tile_pool(name="defer", bufs=1) as defer_pool,
            tc.tile_pool(name="tmp", bufs=4) as tmp_pool,
            tc.tile_pool(name="ostage", bufs=5) as out_pool,
            tc.tile_pool(name="mm_ps", bufs=3, space="PSUM") as mm_ps_pool,
            tc.tile_pool(name="ry_ps", bufs=1, space="PSUM") as ry_ps_pool,
            tc.tile_pool(name="rx_ps", bufs=1, space="PSUM") as rx_ps_pool,
        ):
            ones = const_pool.tile([P, P], BF16, name="ones")
            nc.vector.memset(ones, 1.0)

            def dummy_mms(n):
                # junk matmuls with no input deps; they run whenever the
                # PE would otherwise idle waiting on a DMA chunk, keeping
                # the HAM activity window busy (no K=4/8 re-throttle).
                for _ in range(n):
                    wps = mm_ps_pool.tile([P, 512], F32, name="wps", tag="ps")
                    nc.tensor.matmul(wps[:, 0:P], lhsT=ones, rhs=ones,
                                     start=True, stop=True)

            dummy_mms(WARMUP)

            # ---- input loads: x on sync, y on ACT queue (parallel DMA
            # engines), 512KB contiguous chunks.
            xt, yt = [], []
            for k in range(KC):
                xk = xin_pool.tile([P, N], BF16, name=f"xt{k}", tag=f"xt{k}")
                yk = yin_pool.tile([P, N], BF16, name=f"yt{k}", tag=f"yt{k}")
                nc.sync.dma_start(out=xk, in_=xT[k * P:(k + 1) * P, :])
                nc.scalar.dma_start(out=yk, in_=yT[k * P:(k + 1) * P, :])
                xt.append(xk)
                yt.append(yk)

            # ---- per-chunk load-phase work, k-grouped ----------------
            ssqx = norm_pool.tile([P, NT], F32, name="ssqx")
            ry_ps = [
                ry_ps_pool.tile([P, 512], F32, name=f"ry_ps{c}", tag=f"ry{c}")
                for c in range(MC)
            ]
            for k in range(KC):
                xs = sq_pool.tile([P, N], BF16, name=f"xsq{k}", tag=f"xsq{k}")
                nc.vector.tensor_tensor(xs, xt[k], xt[k], mult)
                rxk = rx_ps_pool.tile([P, NT], F32, name=f"rx_ps{k}", tag="rx")
                for t in range(NT):
                    nc.tensor.matmul(
                        rxk[:, t:t + 1],
                        lhsT=xs[:, t * P:(t + 1) * P],
                        rhs=ones[:, 0:1],
                        start=True, stop=True,
                    )
                if k == 0:
                    nc.vector.tensor_copy(ssqx, rxk)
                else:
                    nc.vector.tensor_tensor(ssqx, ssqx, rxk,
                                            mybir.AluOpType.add)

                ys = sq_pool.tile([P, N], BF16, name=f"ysq{k}", tag=f"ysq{k}")
                nc.vector.tensor_tensor(ys, yt[k], yt[k], mult)
                for c in range(MC):
                    nc.tensor.matmul(
                        ry_ps[c], lhsT=ones, rhs=ys[:, c * 512:(c + 1) * 512],
                        start=(k == 0), stop=(k == KC - 1),
                    )
                dummy_mms(FILL)

            # ---- 1/sqrt via exp(-0.5*ln(s)); all Lns then all Exps ----
            lnx = norm_pool.tile([P, NT], F32, name="lnx")
            rx = norm_pool.tile([P, NT], F32, name="rx")
            lny = norm_pool.tile([P, N], F32, name="lny")
            ry = norm_pool.tile([P, N], BF16, name="ry")
            nc.scalar.activation(lnx, ssqx, Ln)
            for c in range(MC):
                cs = slice(c * 512, (c + 1) * 512)
                nc.scalar.activation(lny[:, cs], ry_ps[c], Ln)
            nc.scalar.activation(rx, lnx, Exp, scale=-0.5)
            for c in range(MC):
                cs = slice(c * 512, (c + 1) * 512)
                nc.scalar.activation(ry[:, cs], lny[:, cs], Exp, scale=-0.5)

            # ---- main matmuls + split epilogue ------------------------
            # k-inner accumulation, 3 rotating PSUM banks. Drain paths:
            #   defer (first 8 groups): DVE copy -> SBUF, scaled later
            #   ACT path (alternating):  tmp = Copy(ps*rx) bf16 on ACT,
            #                            ot = tmp * ry on DVE (327ns)
            #   DVE path (alternating):  ot = (ps*rx)*ry stt (750ns)
            ots, stage = [], []
            gidx = 0
            for t in range(NT):
                ts_ = slice(t * P, (t + 1) * P)
                ot = out_pool.tile([P, N], BF16, name="ot", tag="ot")
                ots.append(ot)
                for c in range(MC):
                    cs = slice(c * 512, (c + 1) * 512)
                    ps = mm_ps_pool.tile([P, 512], F32, name="ps", tag="ps")
                    for k in range(KC):
                        nc.tensor.matmul(
                            ps, lhsT=xt[k][:, ts_], rhs=yt[k][:, cs],
                            start=(k == 0), stop=(k == KC - 1),
                        )
                    if gidx < DEFER:
                        st = defer_pool.tile([P, 512], F32, name=f"st{gidx}",
                                             tag=f"st{gidx}")
                        nc.vector.tensor_copy(st, ps)
                        stage.append((t, c, st))
                    elif gidx % 2 == 0:
                        tmp = tmp_pool.tile([P, 512], BF16, name="tmp",
                                            tag="tmp")
                        nc.scalar.activation(tmp, ps, Copy,
                                             scale=rx[:, t:t + 1])
                        nc.vector.tensor_tensor(ot[:, cs], tmp, ry[:, cs],
                                                mult)
                    else:
                        nc.vector.scalar_tensor_tensor(
                            ot[:, cs], in0=ps, scalar=rx[:, t:t + 1],
                            in1=ry[:, cs], op0=mult, op1=mult,
                        )
                    gidx += 1
                    if gidx == DEFER:
                        # deferred epilogues: SBUF stt, waits rx/ry only.
                        # The deferred tiles' out-DMAs must be emitted
                        # AFTER these writes (emission order defines the
                        # dependency graph - v6.0 DMA'd unwritten SBUF).
                        for dt_, dc_, st_ in stage:
                            dcs = slice(dc_ * 512, (dc_ + 1) * 512)
                            nc.vector.scalar_tensor_tensor(
                                ots[dt_][:, dcs], in0=st_,
                                scalar=rx[:, dt_:dt_ + 1], in1=ry[:, dcs],
                                op0=mult, op1=mult,
                            )
                        for dt_ in range(DEFER // MC):
                            nc.sync.dma_start(
                                out=out[dt_ * P:(dt_ + 1) * P, :],
                                in_=ots[dt_])
                # contiguous 512KB row-block store on the sync HWDGE
                # queue (its input work ends at ~16us; ACT's queue now
                # carries the ACT-path epilogue compute instead).
                if t >= DEFER // MC:
                    nc.sync.dma_start(out=out[ts_, :], in_=ot)

    nc.compile()
    return nc


def _get_nc(variant: str = "v6") -> bass.Bass:
    if variant not in _CACHED:
        _CACHED[variant] = _build_nc(variant)
    return _CACHED[variant]


def _shard(x: np.ndarray, y: np.ndarray):
    """Host-side prep: cast to bf16 and transpose to [512, 2048]."""
    xq = np.asarray(x, dtype=np.float32).astype(ml_dtypes.bfloat16)
    yq = np.asarray(y, dtype=np.float32).astype(ml_dtypes.bfloat16)
    xTs = np.ascontiguousarray(np.transpose(xq, (0, 2, 1)))
    yTs = np.ascontiguousarray(np.transpose(yq, (0, 2, 1)))
    return [{"xT": xTs[b], "yT": yTs[b]} for b in range(B)]


def _run(x: np.ndarray, y: np.ndarray, variant: str = "v6",
         trace: bool = False):
    """Returns (out [8, 2048, 2048] f32, BassKernelResults)."""
    nc = _get_nc(variant)
    in_maps = _shard(x, y)
    res = run_bass_kernel_spmd(nc, in_maps, core_ids=list(range(B)), trace=trace)
    out = np.stack([res.results[b]["out"].astype(np.float32) for b in range(B)])
    return out, res


def kernel(x: np.ndarray, y: np.ndarray) -> np.ndarray:
    out, _ = _run(x, y)
    return out


# revision 8
# speedup vs baseline: 1.1771x; 1.0137x over previous
"""Batched cosine-similarity matrix (retrieval_knn) on 8 TRN2 NeuronCores.

reference:  out[b, n, m] = <x[b,n,:], y[b,m,:]> / max(||x[b,n]|| * ||y[b,m]||, 1e-8)
shapes:     x, y: [8, 2048, 512] f32  ->  out: [8, 2048, 2048] f32

Sharding: data-parallel over the batch dim - batch b runs on core b.

v6 (v1 133us, v2 106us, v4 97.6us, v5 93.2us):
  - All-bf16 data path; 4MB input split across both HWDGE queues
    (x on sync, y on ACT); bf16 output upcast on host.
  - v5's limiter was the DVE (busy 67us): every drain is a 750ns
    fp32-PSUM stt (PSUM source pins the DVE at 1x mode). v6 splits the
    epilogue: half the (t,c) groups drain on ACT - Copy(ps * rx) with a
    per-partition scale AP into bf16 - and finish with a 327ns bf16 2x
    DVE multiply by ry; the other half keep the single 750ns DVE stt.
    ~44us on each engine instead of 67 on one.
  - First 8 groups defer: plain DVE copy to SBUF staging (no rx/ry
    dependency), scaled later - the PE streams through the norm tail.
  - ry in bf16 (needed for the 2x DVE multiply; adds ~1e-3 rel err).
  - ACT's single table slot: all Lns then all Exps (2 loads + 1 for
    the hoisted first), after v4 measured 6 alternating reloads.
  - rx via per-k N=1 matmuls start=stop=True (PSUM has_written is
    bank-granular), DVE-accumulated; squares on DVE during the load.
  - Dummy warm-up matmuls bridge chunk waits (HAM stays at K=8/8).
"""

import numpy as np
import ml_dtypes

import concourse.bass as bass
import concourse.bacc as bacc
import concourse.mybir as mybir
import concourse.tile as tile
from concourse.bass_utils import run_bass_kernel_spmd

P = 128          # partitions
D = 512          # feature dim (contraction)
N = 2048         # rows of x / y
B = 8            # batch == n_cores
KC = D // P      # 4 k-chunks
NT = N // P      # 16 n-tiles (output partition tiles)
MC = N // 512    # 4 m-chunks (PSUM-bank width)
WARMUP = 36      # initial dummy matmuls (~3.8us @1.2GHz) to flip HAM to 8/8
FILLS = (8, 8, 18, 0)  # dummies per k-group (k2->k3 wait is longest)
DEFER = 8        # groups drained unscaled to SBUF (PE slack over norm tail)

F32 = mybir.dt.float32
BF16 = mybir.dt.bfloat16

_CACHED = {}


def _build_nc(variant: str = "v7") -> bass.Bass:
    """Build the single-core Bass program (same program runs SPMD on 8 cores)."""
    nc = bacc.Bacc(trn_type="TRN2", target_bir_lowering=False, debug=False)

    xT = nc.dram_tensor("xT", [D, N], BF16, kind="ExternalInput").ap()
    yT = nc.dram_tensor("yT", [D, N], BF16, kind="ExternalInput").ap()
    out = nc.dram_tensor("out", [N, N], BF16, kind="ExternalOutput").ap()

    Ln = mybir.ActivationFunctionType.Ln
    Exp = mybir.ActivationFunctionType.Exp
    Copy = mybir.ActivationFunctionType.Copy
    mult = mybir.AluOpType.mult

    with tile.TileContext(nc) as tc:
        with (
            tc.tile_pool(name="consts", bufs=1) as const_pool,
            tc.tile_pool(name="xin", bufs=1) as xin_pool,
            tc.tile_pool(name="yin", bufs=1) as yin_pool,
            tc.tile_pool(name="sq", bufs=1) as sq_pool,
            tc.tile_pool(name="norms", bufs=1) as norm_pool,
            tc.tile_pool(name="defer", bufs=1) as defer_pool,
            tc.tile_pool(name="tmp", bufs=4) as tmp_pool,
            tc.tile_pool(name="ostage", bufs=5) as out_pool,
            tc.tile_pool(name="mm_ps", bufs=3, space="PSUM") as mm_ps_pool,
            tc.tile_pool(name="ry_ps", bufs=1, space="PSUM") as ry_ps_pool,
            tc.tile_pool(name="rx_ps", bufs=1, space="PSUM") as rx_ps_pool,
        ):
            ones = const_pool.tile([P, P], BF16, name="ones")
            nc.vector.memset(ones, 1.0)

            def dummy_mms(n):
                # junk matmuls with no input deps; they run whenever the
                # PE would otherwise idle waiting on a DMA chunk, keeping
                # the HAM activity window busy (no K=4/8 re-throttle).
                for _ in range(n):
                    wps = mm_ps_pool.tile([P, 512], F32, name="wps", tag="ps")
                    nc.tensor.matmul(wps[:, 0:P], lhsT=ones, rhs=ones,
                                     start=True, stop=True)

            dummy_mms(WARMUP)

            # ---- input loads: x on sync, y on ACT queue (parallel DMA
            # engines), 512KB contiguous chunks.
            xt, yt = [], []
            for k in range(KC):
                xk = xin_pool.tile([P, N], BF16, name=f"xt{k}", tag=f"xt{k}")
                yk = yin_pool.tile([P, N], BF16, name=f"yt{k}", tag=f"yt{k}")
                # 3 parallel DMA paths: sync-HWDGE carries x0-x2, ACT-HWDGE
                # y0-y2, and the gpsimd SWDGE queue the k=3 pair (needed
                # last; measured aggregate was ~276GB/s on 2 queues).
                if k == KC - 1:
                    nc.gpsimd.dma_start(out=xk, in_=xT[k * P:(k + 1) * P, :])
                    nc.gpsimd.dma_start(out=yk, in_=yT[k * P:(k + 1) * P, :])
                else:
                    nc.sync.dma_start(out=xk, in_=xT[k * P:(k + 1) * P, :])
                    nc.scalar.dma_start(out=yk, in_=yT[k * P:(k + 1) * P, :])
                xt.append(xk)
                yt.append(yk)

            # ---- per-chunk load-phase work, k-grouped ----------------
            ssqx = norm_pool.tile([P, NT], F32, name="ssqx")
            ry_ps = [
                ry_ps_pool.tile([P, 512], F32, name=f"ry_ps{c}", tag=f"ry{c}")
                for c in range(MC)
            ]
            for k in range(KC):
                xs = sq_pool.tile([P, N], BF16, name=f"xsq{k}", tag=f"xsq{k}")
                nc.vector.tensor_tensor(xs, xt[k], xt[k], mult)
                rxk = rx_ps_pool.tile([P, NT], F32, name=f"rx_ps{k}", tag="rx")
                for t in range(NT):
                    nc.tensor.matmul(
                        rxk[:, t:t + 1],
                        lhsT=xs[:, t * P:(t + 1) * P],
                        rhs=ones[:, 0:1],
                        start=True, stop=True,
                    )
                if k == 0:
                    nc.vector.tensor_copy(ssqx, rxk)
                else:
                    nc.vector.tensor_tensor(ssqx, ssqx, rxk,
                                            mybir.AluOpType.add)

                ys = sq_pool.tile([P, N], BF16, name=f"ysq{k}", tag=f"ysq{k}")
                nc.vector.tensor_tensor(ys, yt[k], yt[k], mult)
                for c in range(MC):
                    nc.tensor.matmul(
                        ry_ps[c], lhsT=ones, rhs=ys[:, c * 512:(c + 1) * 512],
                        start=(k == 0), stop=(k == KC - 1),
                    )
                dummy_mms(FILLS[k])

            # ---- 1/sqrt via exp(-0.5*ln(s)); all Lns then all Exps ----
            lnx = norm_pool.tile([P, NT], F32, name="lnx")
            rx = norm_pool.tile([P, NT], F32, name="rx")
            lny = norm_pool.tile([P, N], F32, name="lny")
            ry = norm_pool.tile([P, N], BF16, name="ry")
            nc.scalar.activation(lnx, ssqx, Ln)
            for c in range(MC):
                cs = slice(c * 512, (c + 1) * 512)
                nc.scalar.activation(lny[:, cs], ry_ps[c], Ln)
            nc.scalar.activation(rx, lnx, Exp, scale=-0.5)
            for c in range(MC):
                cs = slice(c * 512, (c + 1) * 512)
                nc.scalar.activation(ry[:, cs], lny[:, cs], Exp, scale=-0.5)

            # ---- main matmuls + split epilogue ------------------------
            # k-inner accumulation, 3 rotating PSUM banks. Drain paths:
            #   defer (first 8 groups): DVE copy -> SBUF, scaled later
            #   ACT path (alternating):  tmp = Copy(ps*rx) bf16 on ACT,
            #                            ot = tmp * ry on DVE (327ns)
            #   DVE path (alternating):  ot = (ps*rx)*ry stt (750ns)
            ots, stage = [], []
            gidx = 0
            for t in range(NT):
                ts_ = slice(t * P, (t + 1) * P)
                ot = out_pool.tile([P, N], BF16, name="ot", tag="ot")
                ots.append(ot)
                for c in range(MC):
                    cs = slice(c * 512, (c + 1) * 512)
                    ps = mm_ps_pool.tile([P, 512], F32, name="ps", tag="ps")
                    for k in range(KC):
                        nc.tensor.matmul(
                            ps, lhsT=xt[k][:, ts_], rhs=yt[k][:, cs],
                            start=(k == 0), stop=(k == KC - 1),
                        )
                    if gidx < DEFER:
                        st = defer_pool.tile([P, 512], F32, name=f"st{gidx}",
                                             tag=f"st{gidx}")
                        nc.vector.tensor_copy(st, ps)
                        stage.append((t, c, st))
                    elif gidx % 2 == 0:
                        tmp = tmp_pool.tile([P, 512], BF16, name="tmp",
                                            tag="tmp")
                        nc.scalar.activation(tmp, ps, Copy,
                                             scale=rx[:, t:t + 1])
                        nc.vector.tensor_tensor(ot[:, cs], tmp, ry[:, cs],
                                                mult)
                    else:
                        nc.vector.scalar_tensor_tensor(
                            ot[:, cs], in0=ps, scalar=rx[:, t:t + 1],
                            in1=ry[:, cs], op0=mult, op1=mult,
                        )
                    gidx += 1
                    if gidx == DEFER:
                        # deferred epilogues: SBUF stt, waits rx/ry only.
                        # The deferred tiles' out-DMAs must be emitted
                        # AFTER these writes (emission order defines the
                        # dependency graph - v6.0 DMA'd unwritten SBUF).
                        for dt_, dc_, st_ in stage:
                            dcs = slice(dc_ * 512, (dc_ + 1) * 512)
                            nc.vector.scalar_tensor_tensor(
                                ots[dt_][:, dcs], in0=st_,
                                scalar=rx[:, dt_:dt_ + 1], in1=ry[:, dcs],
                                op0=mult, op1=mult,
                            )
                        for dt_ in range(DEFER // MC):
                            nc.sync.dma_start(
                                out=out[dt_ * P:(dt_ + 1) * P, :],
                                in_=ots[dt_])
                # contiguous 512KB row-block store on the sync HWDGE
                # queue (its input work ends at ~16us; ACT's queue now
                # carries the ACT-path epilogue compute instead).
                if t == NT - 1:
                    # last tile: per-chunk stores so the final DMA only
                    # covers 128KB after the last drain (tail shave)
                    for c in range(MC):
                        cs = slice(c * 512, (c + 1) * 512)
                        nc.sync.dma_start(out=out[ts_, cs], in_=ot[:, cs])
                elif t >= DEFER // MC:
                    nc.sync.dma_start(out=out[ts_, :], in_=ot)

    nc.compile()
    return nc


def _get_nc(variant: str = "v7") -> bass.Bass:
    if variant not in _CACHED:
        _CACHED[variant] = _build_nc(variant)
    return _CACHED[variant]


def _shard(x: np.ndarray, y: np.ndarray):
    """Host-side prep: cast to bf16 and transpose to [512, 2048]."""
    xq = np.asarray(x, dtype=np.float32).astype(ml_dtypes.bfloat16)
    yq = np.asarray(y, dtype=np.float32).astype(ml_dtypes.bfloat16)
    xTs = np.ascontiguousarray(np.transpose(xq, (0, 2, 1)))
    yTs = np.ascontiguousarray(np.transpose(yq, (0, 2, 1)))
    return [{"xT": xTs[b], "yT": yTs[b]} for b in range(B)]


def _run(x: np.ndarray, y: np.ndarray, variant: str = "v7",
         trace: bool = False):
    """Returns (out [8, 2048, 2048] f32, BassKernelResults)."""
    nc = _get_nc(variant)
    in_maps = _shard(x, y)
    res = run_bass_kernel_spmd(nc, in_maps, core_ids=list(range(B)), trace=trace)
    out = np.stack([res.results[b]["out"].astype(np.float32) for b in range(B)])
    return out, res


def kernel(x: np.ndarray, y: np.ndarray) -> np.ndarray:
    out, _ = _run(x, y)
    return out


# revision 9
# speedup vs baseline: 1.2140x; 1.0314x over previous
"""Batched cosine-similarity matrix (retrieval_knn) on 8 TRN2 NeuronCores.

reference:  out[b, n, m] = <x[b,n,:], y[b,m,:]> / max(||x[b,n]|| * ||y[b,m]||, 1e-8)
shapes:     x, y: [8, 2048, 512] f32  ->  out: [8, 2048, 2048] f32

Sharding: data-parallel over the batch dim - batch b runs on core b.

v6 (v1 133us, v2 106us, v4 97.6us, v5 93.2us):
  - All-bf16 data path; 4MB input split across both HWDGE queues
    (x on sync, y on ACT); bf16 output upcast on host.
  - v5's limiter was the DVE (busy 67us): every drain is a 750ns
    fp32-PSUM stt (PSUM source pins the DVE at 1x mode). v6 splits the
    epilogue: half the (t,c) groups drain on ACT - Copy(ps * rx) with a
    per-partition scale AP into bf16 - and finish with a 327ns bf16 2x
    DVE multiply by ry; the other half keep the single 750ns DVE stt.
    ~44us on each engine instead of 67 on one.
  - First 8 groups defer: plain DVE copy to SBUF staging (no rx/ry
    dependency), scaled later - the PE streams through the norm tail.
  - ry in bf16 (needed for the 2x DVE multiply; adds ~1e-3 rel err).
  - ACT's single table slot: all Lns then all Exps (2 loads + 1 for
    the hoisted first), after v4 measured 6 alternating reloads.
  - rx via per-k N=1 matmuls start=stop=True (PSUM has_written is
    bank-granular), DVE-accumulated; squares on DVE during the load.
  - Dummy warm-up matmuls bridge chunk waits (HAM stays at K=8/8).
"""

import numpy as np
import ml_dtypes

import concourse.bass as bass
import concourse.bacc as bacc
import concourse.mybir as mybir
import concourse.tile as tile
from concourse.bass_utils import run_bass_kernel_spmd

P = 128          # partitions
D = 512          # feature dim (contraction)
N = 2048         # rows of x / y
B = 8            # batch == n_cores
KC = D // P      # 4 k-chunks
NT = N // P      # 16 n-tiles (output partition tiles)
MC = N // 512    # 4 m-chunks (PSUM-bank width)
WARMUP = 36      # initial dummy matmuls (~3.8us @1.2GHz) to flip HAM to 8/8
FILLS = (8, 8, 12, 0)  # dummies per k-group (k2->k3 wait is longest)
DEFER = 8        # groups drained unscaled to SBUF (PE slack over norm tail)

F32 = mybir.dt.float32
BF16 = mybir.dt.bfloat16

_CACHED = {}


def _build_nc(variant: str = "v8") -> bass.Bass:
    """Build the single-core Bass program (same program runs SPMD on 8 cores)."""
    nc = bacc.Bacc(trn_type="TRN2", target_bir_lowering=False, debug=False)

    xT = nc.dram_tensor("xT", [D, N], BF16, kind="ExternalInput").ap()
    yT = nc.dram_tensor("yT", [D, N], BF16, kind="ExternalInput").ap()
    out = nc.dram_tensor("out", [N, N], BF16, kind="ExternalOutput").ap()

    Ln = mybir.ActivationFunctionType.Ln
    Exp = mybir.ActivationFunctionType.Exp
    Copy = mybir.ActivationFunctionType.Copy
    mult = mybir.AluOpType.mult

    with tile.TileContext(nc) as tc:
        with (
            tc.tile_pool(name="consts", bufs=1) as const_pool,
            tc.tile_pool(name="xin", bufs=1) as xin_pool,
            tc.tile_pool(name="yin", bufs=1) as yin_pool,
            tc.tile_pool(name="sq", bufs=1) as sq_pool,
            tc.tile_pool(name="norms", bufs=1) as norm_pool,
            tc.tile_pool(name="defer", bufs=1) as defer_pool,
            tc.tile_pool(name="tmp", bufs=4) as tmp_pool,
            tc.tile_pool(name="ostage", bufs=5) as out_pool,
            tc.tile_pool(name="mm_ps", bufs=3, space="PSUM") as mm_ps_pool,
            tc.tile_pool(name="ry_ps", bufs=1, space="PSUM") as ry_ps_pool,
            tc.tile_pool(name="rx_ps", bufs=1, space="PSUM") as rx_ps_pool,
        ):
            ones = const_pool.tile([P, P], BF16, name="ones")
            nc.vector.memset(ones, 1.0)

            def dummy_mms(n):
                # junk matmuls with no input deps; they run whenever the
                # PE would otherwise idle waiting on a DMA chunk, keeping
                # the HAM activity window busy (no K=4/8 re-throttle).
                for _ in range(n):
                    wps = mm_ps_pool.tile([P, 512], F32, name="wps", tag="ps")
                    nc.tensor.matmul(wps[:, 0:P], lhsT=ones, rhs=ones,
                                     start=True, stop=True)

            dummy_mms(WARMUP)

            # ---- input loads: x on sync, y on ACT queue (parallel DMA
            # engines), 512KB contiguous chunks.
            xt, yt = [], []
            for k in range(KC):
                xk = xin_pool.tile([P, N], BF16, name=f"xt{k}", tag=f"xt{k}")
                yk = yin_pool.tile([P, N], BF16, name=f"yt{k}", tag=f"yt{k}")
                # 3 parallel DMA paths: sync-HWDGE carries x0-x2, ACT-HWDGE
                # y0-y2, and the gpsimd SWDGE queue the k=3 pair (needed
                # last; measured aggregate was ~276GB/s on 2 queues).
                if k == KC - 1:
                    nc.gpsimd.dma_start(out=xk, in_=xT[k * P:(k + 1) * P, :])
                    nc.gpsimd.dma_start(out=yk, in_=yT[k * P:(k + 1) * P, :])
                else:
                    nc.sync.dma_start(out=xk, in_=xT[k * P:(k + 1) * P, :])
                    nc.scalar.dma_start(out=yk, in_=yT[k * P:(k + 1) * P, :])
                xt.append(xk)
                yt.append(yk)

            # ---- per-chunk load-phase work, k-grouped ----------------
            ssqx = norm_pool.tile([P, NT], F32, name="ssqx")
            ry_ps = [
                ry_ps_pool.tile([P, 512], F32, name=f"ry_ps{c}", tag=f"ry{c}")
                for c in range(MC)
            ]
            # DVE queue order matters: squares for chunk k+1 are emitted
            # BEFORE the rx PSUM read of chunk k, so the square stream is
            # never blocked behind a PE round-trip (v7 lost ~5us/chunk to
            # that FIFO coupling).
            rxks = []
            for k in range(KC):
                xs = sq_pool.tile([P, N], BF16, name=f"xsq{k}", tag=f"xsq{k}")
                nc.vector.tensor_tensor(xs, xt[k], xt[k], mult)
                ys = sq_pool.tile([P, N], BF16, name=f"ysq{k}", tag=f"ysq{k}")
                nc.vector.tensor_tensor(ys, yt[k], yt[k], mult)
                if k > 0:
                    rk = rxks[k - 1]
                    if k == 1:
                        nc.vector.tensor_copy(ssqx, rk[:, 0:NT])
                    else:
                        nc.vector.tensor_tensor(ssqx, ssqx, rk[:, 0:NT],
                                                mybir.AluOpType.add)
                rxk = rx_ps_pool.tile([P, 512], F32, name=f"rx_ps{k}", tag="rx")
                rxks.append(rxk)
                for t in range(NT):
                    nc.tensor.matmul(
                        rxk[:, t:t + 1],
                        lhsT=xs[:, t * P:(t + 1) * P],
                        rhs=ones[:, 0:1],
                        start=True, stop=True,
                    )
                for c in range(MC):
                    nc.tensor.matmul(
                        ry_ps[c], lhsT=ones, rhs=ys[:, c * 512:(c + 1) * 512],
                        start=(k == 0), stop=(k == KC - 1),
                    )
                dummy_mms(FILLS[k])
            nc.vector.tensor_tensor(ssqx, ssqx, rxks[KC - 1][:, 0:NT],
                                    mybir.AluOpType.add)

            # ---- 1/sqrt via exp(-0.5*ln(s)); all Lns then all Exps ----
            lnx = norm_pool.tile([P, NT], F32, name="lnx")
            rx = norm_pool.tile([P, NT], F32, name="rx")
            lny = norm_pool.tile([P, N], F32, name="lny")
            ry = norm_pool.tile([P, N], BF16, name="ry")
            nc.scalar.activation(lnx, ssqx, Ln)
            for c in range(MC):
                cs = slice(c * 512, (c + 1) * 512)
                nc.scalar.activation(lny[:, cs], ry_ps[c], Ln)
            nc.scalar.activation(rx, lnx, Exp, scale=-0.5)
            for c in range(MC):
                cs = slice(c * 512, (c + 1) * 512)
                nc.scalar.activation(ry[:, cs], lny[:, cs], Exp, scale=-0.5)

            # ---- main matmuls + split epilogue ------------------------
            # k-inner accumulation, 3 rotating PSUM banks. Drain paths:
            #   defer (first 8 groups): DVE copy -> SBUF, scaled later
            #   ACT path (alternating):  tmp = Copy(ps*rx) bf16 on ACT,
            #                            ot = tmp * ry on DVE (327ns)
            #   DVE path (alternating):  ot = (ps*rx)*ry stt (750ns)
            ots, stage = [], []
            gidx = 0
            for t in range(NT):
                ts_ = slice(t * P, (t + 1) * P)
                ot = out_pool.tile([P, N], BF16, name="ot", tag="ot")
                ots.append(ot)
                for c in range(MC):
                    cs = slice(c * 512, (c + 1) * 512)
                    # after the load phase the ry/rx banks are dead: rotate
                    # main groups over all 8 PSUM banks so drain-semaphore
                    # latency stops pacing the MM stream (3-bank rotation
                    # measured ~250ns/MM vs the 213ns fill rate).
                    if gidx < DEFER:
                        ps = mm_ps_pool.tile([P, 512], F32, name="ps",
                                             tag="ps")
                    else:
                        sel = (gidx - DEFER) % 8
                        if sel < 3:
                            ps = mm_ps_pool.tile([P, 512], F32, name="ps",
                                                 tag="ps")
                        elif sel < 7:
                            ps = ry_ps_pool.tile([P, 512], F32, name="ps",
                                                 tag=f"ry{sel - 3}")
                        else:
                            ps = rx_ps_pool.tile([P, 512], F32, name="ps",
                                                 tag="rx")
                    for k in range(KC):
                        nc.tensor.matmul(
                            ps, lhsT=xt[k][:, ts_], rhs=yt[k][:, cs],
                            start=(k == 0), stop=(k == KC - 1),
                        )
                    if gidx < DEFER:
                        st = defer_pool.tile([P, 512], F32, name=f"st{gidx}",
                                             tag=f"st{gidx}")
                        nc.vector.tensor_copy(st, ps)
                        stage.append((t, c, st))
                    elif gidx % 2 == 0:
                        tmp = tmp_pool.tile([P, 512], BF16, name="tmp",
                                            tag="tmp")
                        nc.scalar.activation(tmp, ps, Copy,
                                             scale=rx[:, t:t + 1])
                        nc.vector.tensor_tensor(ot[:, cs], tmp, ry[:, cs],
                                                mult)
                    else:
                        nc.vector.scalar_tensor_tensor(
                            ot[:, cs], in0=ps, scalar=rx[:, t:t + 1],
                            in1=ry[:, cs], op0=mult, op1=mult,
                        )
                    gidx += 1
                    if gidx == DEFER:
                        # deferred epilogues: SBUF stt, waits rx/ry only.
                        # The deferred tiles' out-DMAs must be emitted
                        # AFTER these writes (emission order defines the
                        # dependency graph - v6.0 DMA'd unwritten SBUF).
                        for dt_, dc_, st_ in stage:
                            dcs = slice(dc_ * 512, (dc_ + 1) * 512)
                            nc.vector.scalar_tensor_tensor(
                                ots[dt_][:, dcs], in0=st_,
                                scalar=rx[:, dt_:dt_ + 1], in1=ry[:, dcs],
                                op0=mult, op1=mult,
                            )
                        for dt_ in range(DEFER // MC):
                            nc.sync.dma_start(
                                out=out[dt_ * P:(dt_ + 1) * P, :],
                                in_=ots[dt_])
                # contiguous 512KB row-block store on the sync HWDGE
                # queue (its input work ends at ~16us; ACT's queue now
                # carries the ACT-path epilogue compute instead).
                if t == NT - 1:
                    # last tile: per-chunk stores so the final DMA only
                    # covers 128KB after the last drain (tail shave)
                    for c in range(MC):
                        cs = slice(c * 512, (c + 1) * 512)
                        nc.sync.dma_start(out=out[ts_, cs], in_=ot[:, cs])
                elif t >= DEFER // MC:
                    nc.sync.dma_start(out=out[ts_, :], in_=ot)

    nc.compile()
    return nc


def _get_nc(variant: str = "v8") -> bass.Bass:
    if variant not in _CACHED:
        _CACHED[variant] = _build_nc(variant)
    return _CACHED[variant]


def _shard(x: np.ndarray, y: np.ndarray):
    """Host-side prep: cast to bf16 and transpose to [512, 2048]."""
    xq = np.asarray(x, dtype=np.float32).astype(ml_dtypes.bfloat16)
    yq = np.asarray(y, dtype=np.float32).astype(ml_dtypes.bfloat16)
    xTs = np.ascontiguousarray(np.transpose(xq, (0, 2, 1)))
    yTs = np.ascontiguousarray(np.transpose(yq, (0, 2, 1)))
    return [{"xT": xTs[b], "yT": yTs[b]} for b in range(B)]


def _run(x: np.ndarray, y: np.ndarray, variant: str = "v8",
         trace: bool = False):
    """Returns (out [8, 2048, 2048] f32, BassKernelResults)."""
    nc = _get_nc(variant)
    in_maps = _shard(x, y)
    res = run_bass_kernel_spmd(nc, in_maps, core_ids=list(range(B)), trace=trace)
    out = np.stack([res.results[b]["out"].astype(np.float32) for b in range(B)])
    return out, res


def kernel(x: np.ndarray, y: np.ndarray) -> np.ndarray:
    out, _ = _run(x, y)
    return out


# revision 10
# speedup vs baseline: 1.2208x; 1.0056x over previous
"""Batched cosine-similarity matrix (retrieval_knn) on 8 TRN2 NeuronCores.

reference:  out[b, n, m] = <x[b,n,:], y[b,m,:]> / max(||x[b,n]|| * ||y[b,m]||, 1e-8)
shapes:     x, y: [8, 2048, 512] f32  ->  out: [8, 2048, 2048] f32

Sharding: data-parallel over the batch dim - batch b runs on core b.

v6 (v1 133us, v2 106us, v4 97.6us, v5 93.2us):
  - All-bf16 data path; 4MB input split across both HWDGE queues
    (x on sync, y on ACT); bf16 output upcast on host.
  - v5's limiter was the DVE (busy 67us): every drain is a 750ns
    fp32-PSUM stt (PSUM source pins the DVE at 1x mode). v6 splits the
    epilogue: half the (t,c) groups drain on ACT - Copy(ps * rx) with a
    per-partition scale AP into bf16 - and finish with a 327ns bf16 2x
    DVE multiply by ry; the other half keep the single 750ns DVE stt.
    ~44us on each engine instead of 67 on one.
  - First 8 groups defer: plain DVE copy to SBUF staging (no rx/ry
    dependency), scaled later - the PE streams through the norm tail.
  - ry in bf16 (needed for the 2x DVE multiply; adds ~1e-3 rel err).
  - ACT's single table slot: all Lns then all Exps (2 loads + 1 for
    the hoisted first), after v4 measured 6 alternating reloads.
  - rx via per-k N=1 matmuls start=stop=True (PSUM has_written is
    bank-granular), DVE-accumulated; squares on DVE during the load.
  - Dummy warm-up matmuls bridge chunk waits (HAM stays at K=8/8).
"""

import numpy as np
import ml_dtypes

import concourse.bass as bass
import concourse.bacc as bacc
import concourse.mybir as mybir
import concourse.tile as tile
from concourse.bass_utils import run_bass_kernel_spmd

P = 128          # partitions
D = 512          # feature dim (contraction)
N = 2048         # rows of x / y
B = 8            # batch == n_cores
KC = D // P      # 4 k-chunks
NT = N // P      # 16 n-tiles (output partition tiles)
MC = N // 512    # 4 m-chunks (PSUM-bank width)
WARMUP = 36      # initial dummy matmuls (~3.8us @1.2GHz) to flip HAM to 8/8
FILLS = (6, 6, 8, 0)  # dummies per k-group (k2->k3 wait is longest)
DEFER = 8        # groups drained unscaled to SBUF (PE slack over norm tail)

F32 = mybir.dt.float32
BF16 = mybir.dt.bfloat16

_CACHED = {}


def _build_nc(variant: str = "v9") -> bass.Bass:
    """Build the single-core Bass program (same program runs SPMD on 8 cores)."""
    nc = bacc.Bacc(trn_type="TRN2", target_bir_lowering=False, debug=False)

    xT = nc.dram_tensor("xT", [D, N], BF16, kind="ExternalInput").ap()
    yT = nc.dram_tensor("yT", [D, N], BF16, kind="ExternalInput").ap()
    out = nc.dram_tensor("out", [N, N], BF16, kind="ExternalOutput").ap()

    Ln = mybir.ActivationFunctionType.Ln
    Exp = mybir.ActivationFunctionType.Exp
    Copy = mybir.ActivationFunctionType.Copy
    mult = mybir.AluOpType.mult

    with tile.TileContext(nc) as tc:
        with (
            tc.tile_pool(name="consts", bufs=1) as const_pool,
            tc.tile_pool(name="xin", bufs=1) as xin_pool,
            tc.tile_pool(name="yin", bufs=1) as yin_pool,
            tc.tile_pool(name="sq", bufs=1) as sq_pool,
            tc.tile_pool(name="norms", bufs=1) as norm_pool,
            tc.tile_pool(name="defer", bufs=1) as defer_pool,
            tc.tile_pool(name="tmp", bufs=4) as tmp_pool,
            tc.tile_pool(name="ostage", bufs=5) as out_pool,
            tc.tile_pool(name="mm_ps", bufs=3, space="PSUM") as mm_ps_pool,
            tc.tile_pool(name="ry_ps", bufs=1, space="PSUM") as ry_ps_pool,
            tc.tile_pool(name="rx_ps", bufs=1, space="PSUM") as rx_ps_pool,
        ):
            ones = const_pool.tile([P, P], BF16, name="ones")
            nc.vector.memset(ones, 1.0)

            def dummy_mms(n):
                # junk matmuls with no input deps; they run whenever the
                # PE would otherwise idle waiting on a DMA chunk, keeping
                # the HAM activity window busy (no K=4/8 re-throttle).
                for _ in range(n):
                    wps = mm_ps_pool.tile([P, 512], F32, name="wps", tag="ps")
                    nc.tensor.matmul(wps[:, 0:P], lhsT=ones, rhs=ones,
                                     start=True, stop=True)

            dummy_mms(WARMUP)

            # ---- input loads: x on sync, y on ACT queue (parallel DMA
            # engines), 512KB contiguous chunks.
            xt, yt = [], []
            for k in range(KC):
                xk = xin_pool.tile([P, N], BF16, name=f"xt{k}", tag=f"xt{k}")
                yk = yin_pool.tile([P, N], BF16, name=f"yt{k}", tag=f"yt{k}")
                # 3 parallel DMA paths balanced by measured bandwidth:
                # sync/ACT HWDGE ~155GB/s each, gpsimd SWDGE ~45GB/s.
                # gpsimd gets only the trailing halves of the k=3 pair
                # (512KB) so all three queues finish at ~the same time.
                ks = slice(k * P, (k + 1) * P)
                if k == KC - 1:
                    nc.sync.dma_start(out=xk[:, 0:1024], in_=xT[ks, 0:1024])
                    nc.scalar.dma_start(out=yk[:, 0:1024], in_=yT[ks, 0:1024])
                    nc.gpsimd.dma_start(out=xk[:, 1024:N], in_=xT[ks, 1024:N])
                    nc.gpsimd.dma_start(out=yk[:, 1024:N], in_=yT[ks, 1024:N])
                else:
                    nc.sync.dma_start(out=xk, in_=xT[ks, :])
                    nc.scalar.dma_start(out=yk, in_=yT[ks, :])
                xt.append(xk)
                yt.append(yk)

            # ---- per-chunk load-phase work, k-grouped ----------------
            ssqx = norm_pool.tile([P, NT], F32, name="ssqx")
            ry_ps = [
                ry_ps_pool.tile([P, 512], F32, name=f"ry_ps{c}", tag=f"ry{c}")
                for c in range(MC)
            ]
            # DVE queue order matters: squares for chunk k+1 are emitted
            # BEFORE the rx PSUM read of chunk k, so the square stream is
            # never blocked behind a PE round-trip (v7 lost ~5us/chunk to
            # that FIFO coupling).
            rxks = []
            for k in range(KC):
                xs = sq_pool.tile([P, N], BF16, name=f"xsq{k}", tag=f"xsq{k}")
                nc.vector.tensor_tensor(xs, xt[k], xt[k], mult)
                ys = sq_pool.tile([P, N], BF16, name=f"ysq{k}", tag=f"ysq{k}")
                nc.vector.tensor_tensor(ys, yt[k], yt[k], mult)
                if k > 0:
                    rk = rxks[k - 1]
                    if k == 1:
                        nc.vector.tensor_copy(ssqx, rk[:, 0:NT])
                    else:
                        nc.vector.tensor_tensor(ssqx, ssqx, rk[:, 0:NT],
                                                mybir.AluOpType.add)
                rxk = rx_ps_pool.tile([P, 512], F32, name=f"rx_ps{k}", tag="rx")
                rxks.append(rxk)
                for t in range(NT):
                    nc.tensor.matmul(
                        rxk[:, t:t + 1],
                        lhsT=xs[:, t * P:(t + 1) * P],
                        rhs=ones[:, 0:1],
                        start=True, stop=True,
                    )
                for c in range(MC):
                    nc.tensor.matmul(
                        ry_ps[c], lhsT=ones, rhs=ys[:, c * 512:(c + 1) * 512],
                        start=(k == 0), stop=(k == KC - 1),
                    )
                dummy_mms(FILLS[k])
            nc.vector.tensor_tensor(ssqx, ssqx, rxks[KC - 1][:, 0:NT],
                                    mybir.AluOpType.add)

            # ---- 1/sqrt via exp(-0.5*ln(s)); all Lns then all Exps ----
            lnx = norm_pool.tile([P, NT], F32, name="lnx")
            rx = norm_pool.tile([P, NT], F32, name="rx")
            lny = norm_pool.tile([P, N], F32, name="lny")
            ry = norm_pool.tile([P, N], BF16, name="ry")
            nc.scalar.activation(lnx, ssqx, Ln)
            for c in range(MC):
                cs = slice(c * 512, (c + 1) * 512)
                nc.scalar.activation(lny[:, cs], ry_ps[c], Ln)
            nc.scalar.activation(rx, lnx, Exp, scale=-0.5)
            for c in range(MC):
                cs = slice(c * 512, (c + 1) * 512)
                nc.scalar.activation(ry[:, cs], lny[:, cs], Exp, scale=-0.5)

            # ---- main matmuls + split epilogue ------------------------
            # k-inner accumulation, 3 rotating PSUM banks. Drain paths:
            #   defer (first 8 groups): DVE copy -> SBUF, scaled later
            #   ACT path (alternating):  tmp = Copy(ps*rx) bf16 on ACT,
            #                            ot = tmp * ry on DVE (327ns)
            #   DVE path (alternating):  ot = (ps*rx)*ry stt (750ns)
            ots, stage = [], []
            gidx = 0
            for t in range(NT):
                ts_ = slice(t * P, (t + 1) * P)
                ot = out_pool.tile([P, N], BF16, name="ot", tag="ot")
                ots.append(ot)
                for c in range(MC):
                    cs = slice(c * 512, (c + 1) * 512)
                    # after the load phase the ry/rx banks are dead: rotate
                    # main groups over all 8 PSUM banks so drain-semaphore
                    # latency stops pacing the MM stream (3-bank rotation
                    # measured ~250ns/MM vs the 213ns fill rate).
                    if gidx < DEFER:
                        ps = mm_ps_pool.tile([P, 512], F32, name="ps",
                                             tag="ps")
                    else:
                        sel = (gidx - DEFER) % 8
                        if sel < 3:
                            ps = mm_ps_pool.tile([P, 512], F32, name="ps",
                                                 tag="ps")
                        elif sel < 7:
                            ps = ry_ps_pool.tile([P, 512], F32, name="ps",
                                                 tag=f"ry{sel - 3}")
                        else:
                            ps = rx_ps_pool.tile([P, 512], F32, name="ps",
                                                 tag="rx")
                    for k in range(KC):
                        nc.tensor.matmul(
                            ps, lhsT=xt[k][:, ts_], rhs=yt[k][:, cs],
                            start=(k == 0), stop=(k == KC - 1),
                        )
                    if gidx < DEFER:
                        st = defer_pool.tile([P, 512], F32, name=f"st{gidx}",
                                             tag=f"st{gidx}")
                        nc.vector.tensor_copy(st, ps)
                        stage.append((t, c, st))
                    elif gidx % 2 == 0:
                        tmp = tmp_pool.tile([P, 512], BF16, name="tmp",
                                            tag="tmp")
                        nc.scalar.activation(tmp, ps, Copy,
                                             scale=rx[:, t:t + 1])
                        nc.vector.tensor_tensor(ot[:, cs], tmp, ry[:, cs],
                                                mult)
                    else:
                        nc.vector.scalar_tensor_tensor(
                            ot[:, cs], in0=ps, scalar=rx[:, t:t + 1],
                            in1=ry[:, cs], op0=mult, op1=mult,
                        )
                    gidx += 1
                    if gidx == DEFER:
                        # deferred epilogues: SBUF stt, waits rx/ry only.
                        # The deferred tiles' out-DMAs must be emitted
                        # AFTER these writes (emission order defines the
                        # dependency graph - v6.0 DMA'd unwritten SBUF).
                        for dt_, dc_, st_ in stage:
                            dcs = slice(dc_ * 512, (dc_ + 1) * 512)
                            nc.vector.scalar_tensor_tensor(
                                ots[dt_][:, dcs], in0=st_,
                                scalar=rx[:, dt_:dt_ + 1], in1=ry[:, dcs],
                                op0=mult, op1=mult,
                            )
                        for dt_ in range(DEFER // MC):
                            nc.sync.dma_start(
                                out=out[dt_ * P:(dt_ + 1) * P, :],
                                in_=ots[dt_])
                # contiguous 512KB row-block store on the sync HWDGE
                # queue (its input work ends at ~16us; ACT's queue now
                # carries the ACT-path epilogue compute instead).
                if t == NT - 1:
                    # last tile: per-chunk stores so the final DMA only
                    # covers 128KB after the last drain (tail shave)
                    for c in range(MC):
                        cs = slice(c * 512, (c + 1) * 512)
                        nc.sync.dma_start(out=out[ts_, cs], in_=ot[:, cs])
                elif t >= DEFER // MC:
                    nc.sync.dma_start(out=out[ts_, :], in_=ot)

    nc.compile()
    return nc


def _get_nc(variant: str = "v9") -> bass.Bass:
    if variant not in _CACHED:
        _CACHED[variant] = _build_nc(variant)
    return _CACHED[variant]


def _shard(x: np.ndarray, y: np.ndarray):
    """Host-side prep: cast to bf16 and transpose to [512, 2048]."""
    xq = np.asarray(x, dtype=np.float32).astype(ml_dtypes.bfloat16)
    yq = np.asarray(y, dtype=np.float32).astype(ml_dtypes.bfloat16)
    xTs = np.ascontiguousarray(np.transpose(xq, (0, 2, 1)))
    yTs = np.ascontiguousarray(np.transpose(yq, (0, 2, 1)))
    return [{"xT": xTs[b], "yT": yTs[b]} for b in range(B)]


def _run(x: np.ndarray, y: np.ndarray, variant: str = "v9",
         trace: bool = False):
    """Returns (out [8, 2048, 2048] f32, BassKernelResults)."""
    nc = _get_nc(variant)
    in_maps = _shard(x, y)
    res = run_bass_kernel_spmd(nc, in_maps, core_ids=list(range(B)), trace=trace)
    out = np.stack([res.results[b]["out"].astype(np.float32) for b in range(B)])
    return out, res


def kernel(x: np.ndarray, y: np.ndarray) -> np.ndarray:
    out, _ = _run(x, y)
    return out
